# revision 22
# baseline (speedup 1.0000x reference)
"""Trainium2 Bass kernel for AttnPooling (segment softmax pooling).

kernel(**inputs) takes the FULL inputs (H [N,128] f32, w1 [64,128],
w2 [1,64], batch [N] int64 sorted, size scalar) and returns the FULL
[size, 128] f32 output, computed on 8 NeuronCores via bass/Tile.

Self-contained: includes the wait-legalization/ACT-table/NTFF shims the
plain-Bass + TileContext + walrus flow needs in this environment.
"""

import contextlib
import ctypes
import sys
import types

import numpy as np

import concourse.bass as bass
import concourse.tile as tile
from concourse import mybir
from concourse.bass_utils import run_bass_kernel_spmd


# ---------------------------------------------------------------------------
# environment shims
def legalize_waits(nc):
    br = bass._bass_rust
    try:
        br.move_matmul_waits_to_ldweights(nc.m)
    except Exception:
        pass
    br.generate_event_semaphores(nc)
    return nc


def insert_act_tables(nc):
    has_activation = any(
        isinstance(i, mybir.InstActivation)
        for b in nc.main_func.blocks
        for i in b.instructions
    )
    if not has_activation:
        return
    from concourse.hw_specs import get_activation_tables
    tables = list(get_activation_tables(nc.m.arch).items())
    bass._bass_rust.insert_act_table_loads(nc, tables)


def install_ntff_hook(so_path="/opt/axon/libaxon_pjrt.so"):
    if "antenv.axon_hooks" in sys.modules:
        return
    try:
        lib = ctypes.CDLL(so_path)
    except OSError:
        return
    if not hasattr(lib, "axon_start_nrt_profile"):
        return
    lib.axon_start_nrt_profile.argtypes = [
        ctypes.POINTER(ctypes.c_int64), ctypes.c_size_t]
    lib.axon_start_nrt_profile.restype = ctypes.c_int64
    lib.axon_stop_nrt_profile.argtypes = [ctypes.c_char_p]
    lib.axon_stop_nrt_profile.restype = ctypes.c_int64

    @contextlib.contextmanager
    def _hook(output_dir, device_ids):
        import jax
        jax.devices()
        if device_ids:
            ids = (ctypes.c_int64 * len(device_ids))(*device_ids)
            rc = lib.axon_start_nrt_profile(ids, len(device_ids))
        else:
            rc = lib.axon_start_nrt_profile(None, 0)
        if rc != 0:
            raise RuntimeError(f"axon_start_nrt_profile rc={rc}")
        try:
            yield
        finally:
            n = lib.axon_stop_nrt_profile(str(output_dir).encode())
            print(f"ntff profile: {n} file(s) written to {output_dir}",
                  file=sys.stderr)

    mod = types.ModuleType("antenv.axon_hooks")
    mod._hook = _hook
    mod.get_axon_ntff_profile_hook = lambda: _hook
    mod.set_axon_ntff_profile_hook = lambda h: None
    sys.modules["antenv.axon_hooks"] = mod


install_ntff_hook()


F32 = mybir.dt.float32
F16 = mybir.dt.float16

P = 128
IN = 128
HID = 64
BLK = 16          # tiles per block
GRP = 4           # tiles per MLP group
PAIR = 2 * GRP    # tiles per DMA / tanh / exp batch


def build_nc(TPC: int, W: int, pack_mm1: bool = True):
    assert TPC % BLK == 0
    n_blocks = TPC // BLK
    n_pairs = TPC // PAIR
    PW = PAIR * IN + PAIR * W      # fp16 columns per pair payload
    nc = bass.Bass("TRN2", target_bir_lowering=False, debug=False)

    HP = nc.dram_tensor("HP", [n_pairs, P, PW], F16, kind="ExternalInput")
    w1T = nc.dram_tensor("w1T", [IN, HID], F16, kind="ExternalInput")
    w2T = nc.dram_tensor("w2T", [2 * HID, 2], F16, kind="ExternalInput")
    ident = nc.dram_tensor("ident", [P, P], F16, kind="ExternalInput")
    parts = nc.dram_tensor("parts", [P, n_blocks * W], F32,
                           kind="ExternalOutput")
    e_out = nc.dram_tensor("e_out", [P, TPC], F16, kind="ExternalOutput")

    with tile.TileContext(nc) as tc:
        with (
            tc.tile_pool(name="consts", bufs=1) as consts,
            tc.tile_pool(name="hin", bufs=8) as hin,
            tc.tile_pool(name="ht", bufs=3) as htp,
            tc.tile_pool(name="xt", bufs=3) as xtp,
            tc.tile_pool(name="cmp", bufs=8) as cmp_p,
            tc.tile_pool(name="ps_tr", bufs=1, space="PSUM") as ps_tr,
            tc.tile_pool(name="ps_mlp", bufs=2, space="PSUM") as ps_mlp,
            tc.tile_pool(name="ps_s", bufs=2, space="PSUM") as ps_s,
            tc.tile_pool(name="ps_blk", bufs=2, space="PSUM") as ps_blk,
        ):
            w1T_sb = consts.tile([IN, HID], F16)
            nc.sync.dma_start(w1T_sb[:], w1T[:])
            w2z_sb = consts.tile([2 * HID, 2], F16)
            nc.sync.dma_start(w2z_sb[:], w2T[:])
            ident_sb = consts.tile([P, P], F16)
            nc.sync.dma_start(ident_sb[:], ident[:])
            e_buf = consts.tile([P, TPC], F16)
            parts_buf = consts.tile([P, n_blocks * W], F32)

            # PE warmup: ~3.5us of dense matmul activity releases the HAM
            # clock throttle (1.2 -> 2.4 GHz) before the steady state.
            warm = ps_tr.tile([P, 2 * GRP * P], F32, tag="trp")
            for _ in range(48):
                nc.tensor.matmul(warm[:, :P], ident_sb[:], ident_sb[:],
                                 start=True, stop=True)

            for b in range(n_blocks):
                blockp = ps_blk.tile([P, W], F32, tag="blockp")
                for gp in range(BLK // PAIR):
                    pi = b * (BLK // PAIR) + gp
                    tp0 = b * BLK + gp * PAIR
                    hp = hin.tile([P, PW], F16, tag="hin")
                    nc.sync.dma_start(hp[:], HP[pi])

                    mlp = ps_mlp.tile([2 * HID, GRP * P], F32, tag="mlp")
                    scol = ps_s.tile([P, PAIR], F32, tag="scol")
                    # both groups' transposes land in one 2-bank psum tile;
                    # ONE strided ACT copy + ONE strided DVE cast each span
                    # both groups, so the two col-packed mm1s share an
                    # identical dependency set and issue back-to-back
                    # (concurrent h0/h64 execution), with 4->2 copy
                    # instructions saving fixed overhead.
                    trp2 = ps_tr.tile([P, 2 * GRP * P], F32, tag="trp",
                                      name="trp2")
                    HT2 = htp.tile([P, 2 * GRP * P], F16, tag="HT",
                                   name="HT2")
                    for g in range(2):
                        for j in range(GRP):
                            t_off = (g * GRP + j) * IN
                            tp = (g * GRP + j) * P
                            nc.tensor.matmul(
                                trp2[:, tp:tp + P],
                                hp[:, t_off:t_off + IN],
                                ident_sb[:],
                                start=True,
                                stop=True,
                            )
                    act_cols = 256
                    t2v = trp2[:].rearrange("p (g c) -> p g c", g=2)
                    h2v = HT2[:].rearrange("p (g c) -> p g c", g=2)
                    nc.scalar.copy(h2v[:, :, :act_cols],
                                   t2v[:, :, :act_cols])
                    nc.vector.tensor_copy(h2v[:, :, act_cols:],
                                          t2v[:, :, act_cols:])

                    for g in range(2):
                        pos = (0, g * HID) if pack_mm1 else None
                        nc.tensor.matmul(
                            mlp[g * HID:(g + 1) * HID, :],
                            w1T_sb[:],
                            HT2[:, g * GRP * P:(g + 1) * GRP * P],
                            start=True,
                            stop=True,
                            tile_position=pos,
                        )

                    xT = xtp.tile([2 * HID, GRP * P], F16, tag="xT")
                    nc.scalar.activation(
                        xT[:], mlp[:], mybir.ActivationFunctionType.Tanh
                    )

                    # one matmul per tile-pair column: lhsT = full [128, 128]
                    # xT slice stacks both groups' hidden units on K;
                    # rhs = [[w2;0],[0;w2]] -> scores for tiles j and j+4,
                    # written via a stride-4 column view so scol stays in
                    # tile order.
                    scolv = scol[:].rearrange("p (a b) -> p b a", b=GRP)
                    for j in range(GRP):
                        nc.tensor.matmul(
                            scolv[:, j, :],
                            xT[:, j * P:(j + 1) * P],
                            w2z_sb[:],
                            start=True,
                            stop=True,
                        )
                    nc.scalar.activation(
                        e_buf[:, tp0:tp0 + PAIR],
                        scol[:],
                        mybir.ActivationFunctionType.Exp,
                    )

                    # one fused ce build for the whole pair:
                    # ce[p, k*W+w] = cmp01[p, k, w] * e[p, k]
                    c0 = PAIR * IN
                    ce = cmp_p.tile([P, PAIR * W], F16, tag="ce")
                    nc.vector.tensor_tensor(
                        ce[:].rearrange("p (k w) -> p k w", k=PAIR),
                        hp[:, c0:c0 + PAIR * W].rearrange(
                            "p (k w) -> p k w", k=PAIR),
                        e_buf[:, tp0:tp0 + PAIR][:, :, None].broadcast_to(
                            [P, PAIR, W]),
                        op=mybir.AluOpType.mult,
                    )
                    for k in range(PAIR):
                        first = (gp == 0 and k == 0)
                        last = (gp == BLK // PAIR - 1 and k == PAIR - 1)
                        nc.tensor.matmul(
                            blockp[:],
                            hp[:, k * IN:(k + 1) * IN],
                            ce[:, k * W:(k + 1) * W],
                            start=first,
                            stop=last,
                        )

                nc.vector.tensor_copy(
                    parts_buf[:, b * W:(b + 1) * W], blockp[:]
                )

            nc.sync.dma_start(e_out[:], e_buf[:])
            nc.sync.dma_start(parts[:], parts_buf[:])

    insert_act_tables(nc)
    legalize_waits(nc)
    return nc


def prep_inputs(H, w1, w2, batch, G, TPC, W, n_cores):
    """Host-side input prep. Returns (in_maps, bases, meta)."""
    N = H.shape[0]
    rows_per_core = TPC * P
    total_rows = n_cores * rows_per_core
    assert total_rows >= N, (total_rows, N)

    batch = np.asarray(batch, dtype=np.int64)
    batch_pad = np.full(total_rows, -100, dtype=np.int64)
    batch_pad[:N] = batch

    n_blocks = TPC // BLK
    block_rows = BLK * P
    bases = np.zeros((n_cores, n_blocks), dtype=np.int64)
    for c in range(n_cores):
        for b in range(n_blocks):
            r0 = c * rows_per_core + b * block_rows
            bases[c, b] = batch_pad[r0] if r0 < N else 0
    for c in range(n_cores):
        for b in range(n_blocks):
            r0 = c * rows_per_core + b * block_rows
            r1 = min(r0 + block_rows, N)
            if r0 < N:
                span = batch_pad[r0:r1].max() - bases[c, b] + 1
                assert span <= W, f"block span {span} > W={W} at core {c} block {b}"

    bsh = batch_pad - np.repeat(bases.reshape(-1), block_rows)[:total_rows]
    # host-built one-hot window indicator [total_rows, W] fp16
    cmp01 = (bsh[:, None] == np.arange(W)[None, :]).astype(np.float16)

    H_pad = np.zeros((total_rows, IN), dtype=np.float16)
    H_pad[:N] = H.astype(np.float16)

    w1T = np.ascontiguousarray(w1.T).astype(np.float16)    # [128, 64]
    # block-diagonal w2 selector: [128, 2] = [[w2, 0], [0, w2]]
    w2T = np.zeros((2 * HID, 2), dtype=np.float16)
    w2T[:HID, 0] = w2[0].astype(np.float16)
    w2T[HID:, 1] = w2[0].astype(np.float16)
    ident = np.eye(P, dtype=np.float16)

    n_pairs = TPC // PAIR
    PW = PAIR * IN + PAIR * W
    in_maps = []
    for c in range(n_cores):
        r0, r1 = c * rows_per_core, (c + 1) * rows_per_core
        Hc = H_pad[r0:r1]
        # [n_pairs, 128, PAIR*IN]: HPh[pi, p, k*128+f] = Hc[(pi*8+k)*128+p, f]
        HPh = (Hc.reshape(n_pairs, PAIR, P, IN).transpose(0, 2, 1, 3)
               .reshape(n_pairs, P, PAIR * IN))
        Cc = cmp01[r0:r1]
        CPh = (Cc.reshape(n_pairs, PAIR, P, W).transpose(0, 2, 1, 3)
               .reshape(n_pairs, P, PAIR * W))
        HP = np.ascontiguousarray(
            np.concatenate([HPh, CPh], axis=2)
        )
        in_maps.append({
            "HP": HP,
            "w1T": w1T,
            "w2T": w2T,
            "ident": ident,
        })

    meta = dict(N=N, G=G, TPC=TPC, W=W, n_cores=n_cores,
                rows_per_core=rows_per_core, n_blocks=n_blocks,
                block_rows=block_rows, batch=batch)
    return in_maps, bases, meta


def merge_outputs(results, bases, meta):
    N, G, W = meta["N"], meta["G"], meta["W"]
    n_cores, rows_per_core = meta["n_cores"], meta["rows_per_core"]
    n_blocks, block_rows = meta["n_blocks"], meta["block_rows"]
    batch = meta["batch"]
    total_rows = n_cores * rows_per_core

    out_n = np.zeros((G + 2 * W, IN), dtype=np.float64)
    e_all = np.zeros(total_rows, dtype=np.float64)
    for c in range(n_cores):
        parts = results[c]["parts"]          # [128, n_blocks*W] f32
        e_out = results[c]["e_out"]          # [128, TPC] f16
        e_all[c * rows_per_core:(c + 1) * rows_per_core] = (
            e_out.T.reshape(-1).astype(np.float64)
        )
        for b in range(n_blocks):
            g0 = int(bases[c, b])
            r0 = c * rows_per_core + b * block_rows
            if r0 >= N:
                continue
            out_n[g0:g0 + W] += parts[:, b * W:(b + 1) * W].T.astype(np.float64)

    d = np.bincount(batch, weights=e_all[:N], minlength=G)[:G]
    out_n = out_n[:G]
    with np.errstate(divide="ignore", invalid="ignore"):
        fin = np.where(d[:, None] > 0, out_n / d[:, None], 0.0)
    return fin.astype(np.float32)


def prep_and_run(H, w1, w2, batch, G, TPC=None, W=32, n_cores=8, nc=None,
                 trace=False, pack_mm1=True):
    in_maps, bases, meta = prep_inputs(H, w1, w2, batch, G, TPC, W, n_cores)
    if nc is None:
        nc = build_nc(TPC, W, pack_mm1=pack_mm1)
    res = run_bass_kernel_spmd(
        nc, in_maps, core_ids=list(range(n_cores)), trace=trace
    )
    fin = merge_outputs(res.results, bases, meta)
    return fin, res


_NC_CACHE = {}


def kernel(H, w1, w2, batch, size):
    """Full-input entry point; shards over 8 cores internally."""
    H = np.asarray(H, dtype=np.float32)
    w1 = np.asarray(w1, dtype=np.float32)
    w2 = np.asarray(w2, dtype=np.float32)
    batch = np.asarray(batch).astype(np.int64)
    G = int(np.asarray(size))
    N = H.shape[0]
    n_cores = 8

    # tiles per core: cover N with blocks of BLK tiles
    rows_needed = -(-N // n_cores)
    TPC = -(-rows_needed // (P * BLK)) * BLK

    # window: widest segment span over any block, rounded up with margin
    block_rows = BLK * P
    span = 1
    for r0 in range(0, N, block_rows):
        r1 = min(r0 + block_rows, N)
        span = max(span, int(batch[r1 - 1] - batch[r0]) + 1)
    W = 32
    while W < span:
        W *= 2
    assert W <= 512

    key = (TPC, W)
    if key not in _NC_CACHE:
        _NC_CACHE[key] = build_nc(TPC, W)
    fin, _ = prep_and_run(H, w1, w2, batch, G, TPC=TPC, W=W,
                          n_cores=n_cores, nc=_NC_CACHE[key])
    return fin



# revision 23
# speedup vs baseline: 2.1653x; 2.1653x over previous
"""Trainium2 Bass kernel for AttnPooling (segment softmax pooling).

kernel(**inputs) takes the FULL inputs (H [N,128] f32, w1 [64,128],
w2 [1,64], batch [N] int64 sorted, size scalar) and returns the FULL
[size, 128] f32 output, computed on 8 NeuronCores via bass/Tile.

Self-contained: includes the wait-legalization/ACT-table/NTFF shims the
plain-Bass + TileContext + walrus flow needs in this environment.
"""

import contextlib
import ctypes
import sys
import types

import numpy as np

import concourse.bass as bass
import concourse.tile as tile
from concourse import mybir
from concourse.bass_utils import run_bass_kernel_spmd


# ---------------------------------------------------------------------------
# environment shims
def legalize_waits(nc):
    br = bass._bass_rust
    try:
        br.move_matmul_waits_to_ldweights(nc.m)
    except Exception:
        pass
    br.generate_event_semaphores(nc)
    return nc


def insert_act_tables(nc):
    has_activation = any(
        isinstance(i, mybir.InstActivation)
        for b in nc.main_func.blocks
        for i in b.instructions
    )
    if not has_activation:
        return
    from concourse.hw_specs import get_activation_tables
    tables = list(get_activation_tables(nc.m.arch).items())
    bass._bass_rust.insert_act_table_loads(nc, tables)


def install_ntff_hook(so_path="/opt/axon/libaxon_pjrt.so"):
    if "antenv.axon_hooks" in sys.modules:
        return
    try:
        lib = ctypes.CDLL(so_path)
    except OSError:
        return
    if not hasattr(lib, "axon_start_nrt_profile"):
        return
    lib.axon_start_nrt_profile.argtypes = [
        ctypes.POINTER(ctypes.c_int64), ctypes.c_size_t]
    lib.axon_start_nrt_profile.restype = ctypes.c_int64
    lib.axon_stop_nrt_profile.argtypes = [ctypes.c_char_p]
    lib.axon_stop_nrt_profile.restype = ctypes.c_int64

    @contextlib.contextmanager
    def _hook(output_dir, device_ids):
        import jax
        jax.devices()
        if device_ids:
            ids = (ctypes.c_int64 * len(device_ids))(*device_ids)
            rc = lib.axon_start_nrt_profile(ids, len(device_ids))
        else:
            rc = lib.axon_start_nrt_profile(None, 0)
        if rc != 0:
            raise RuntimeError(f"axon_start_nrt_profile rc={rc}")
        try:
            yield
        finally:
            n = lib.axon_stop_nrt_profile(str(output_dir).encode())
            print(f"ntff profile: {n} file(s) written to {output_dir}",
                  file=sys.stderr)

    mod = types.ModuleType("antenv.axon_hooks")
    mod._hook = _hook
    mod.get_axon_ntff_profile_hook = lambda: _hook
    mod.set_axon_ntff_profile_hook = lambda h: None
    sys.modules["antenv.axon_hooks"] = mod


install_ntff_hook()


F32 = mybir.dt.float32
F16 = mybir.dt.float16

P = 128
IN = 128
HID = 64
BLK = 16          # tiles per block
GRP = 4           # tiles per MLP group
PAIR = 2 * GRP    # tiles per DMA / tanh / exp batch


def build_nc(TPC: int, W: int, pack_mm1: bool = True):
    assert TPC % BLK == 0
    n_blocks = TPC // BLK
    n_pairs = TPC // PAIR
    PW = PAIR * IN + PAIR * W      # fp16 columns per pair payload
    nc = bass.Bass("TRN2", target_bir_lowering=False, debug=False)

    HP = nc.dram_tensor("HP", [n_pairs, P, PW], F16, kind="ExternalInput")
    w1T = nc.dram_tensor("w1T", [IN, HID], F16, kind="ExternalInput")
    w2T = nc.dram_tensor("w2T", [2 * HID, 2], F16, kind="ExternalInput")
    ident = nc.dram_tensor("ident", [P, P], F16, kind="ExternalInput")
    parts = nc.dram_tensor("parts", [P, n_blocks * W], F32,
                           kind="ExternalOutput")
    e_out = nc.dram_tensor("e_out", [P, TPC], F16, kind="ExternalOutput")

    with tile.TileContext(nc) as tc:
        with (
            tc.tile_pool(name="consts", bufs=1) as consts,
            tc.tile_pool(name="hin", bufs=8) as hin,
            tc.tile_pool(name="ht", bufs=4) as htp,
            tc.tile_pool(name="xt", bufs=4) as xtp,
            tc.tile_pool(name="cmp", bufs=8) as cmp_p,
            tc.tile_pool(name="ps_tr", bufs=2, space="PSUM") as ps_tr,
            tc.tile_pool(name="ps_mlp", bufs=2, space="PSUM") as ps_mlp,
            tc.tile_pool(name="ps_s", bufs=1, space="PSUM") as ps_s,
            tc.tile_pool(name="ps_blk", bufs=1, space="PSUM") as ps_blk,
        ):
            w1T_sb = consts.tile([IN, HID], F16)
            nc.sync.dma_start(w1T_sb[:], w1T[:])
            w2z_sb = consts.tile([2 * HID, 2], F16)
            nc.sync.dma_start(w2z_sb[:], w2T[:])
            ident_sb = consts.tile([P, P], F16)
            nc.sync.dma_start(ident_sb[:], ident[:])
            e_buf = consts.tile([P, TPC], F16)
            parts_buf = consts.tile([P, n_blocks * W], F32)

            # PE warmup: ~3.5us of dense matmul activity releases the HAM
            # clock throttle (1.2 -> 2.4 GHz) before the steady state.
            warm = ps_tr.tile([P, 2 * GRP * P], F32, tag="trp")
            for _ in range(48):
                nc.tensor.matmul(warm[:, :P], ident_sb[:], ident_sb[:],
                                 start=True, stop=True)

            for b in range(n_blocks):
                blockp = ps_blk.tile([P, W], F32, tag="blockp")
                for gp in range(BLK // PAIR):
                    pi = b * (BLK // PAIR) + gp
                    tp0 = b * BLK + gp * PAIR
                    hp = hin.tile([P, PW], F16, tag="hin")
                    nc.sync.dma_start(hp[:], HP[pi])

                    mlp = ps_mlp.tile([2 * HID, GRP * P], F32, tag="mlp")
                    scol = ps_s.tile([P, PAIR], F32, tag="scol")
                    # both groups' transposes land in one 2-bank psum tile;
                    # ONE strided ACT copy + ONE strided DVE cast each span
                    # both groups, so the two col-packed mm1s share an
                    # identical dependency set and issue back-to-back
                    # (concurrent h0/h64 execution), with 4->2 copy
                    # instructions saving fixed overhead.
                    trp2 = ps_tr.tile([P, 2 * GRP * P], F32, tag="trp",
                                      name="trp2")
                    HT2 = htp.tile([P, 2 * GRP * P], F16, tag="HT",
                                   name="HT2")
                    for g in range(2):
                        for j in range(GRP):
                            t_off = (g * GRP + j) * IN
                            tp = (g * GRP + j) * P
                            nc.tensor.matmul(
                                trp2[:, tp:tp + P],
                                hp[:, t_off:t_off + IN],
                                ident_sb[:],
                                start=True,
                                stop=True,
                            )
                    act_cols = 224
                    t2v = trp2[:].rearrange("p (g c) -> p g c", g=2)
                    h2v = HT2[:].rearrange("p (g c) -> p g c", g=2)
                    nc.scalar.copy(h2v[:, :, :act_cols],
                                   t2v[:, :, :act_cols])
                    nc.vector.tensor_copy(h2v[:, :, act_cols:],
                                          t2v[:, :, act_cols:])

                    for g in range(2):
                        pos = (0, g * HID) if pack_mm1 else None
                        nc.tensor.matmul(
                            mlp[g * HID:(g + 1) * HID, :],
                            w1T_sb[:],
                            HT2[:, g * GRP * P:(g + 1) * GRP * P],
                            start=True,
                            stop=True,
                            tile_position=pos,
                        )

                    xT = xtp.tile([2 * HID, GRP * P], F16, tag="xT")
                    nc.scalar.activation(
                        xT[:], mlp[:], mybir.ActivationFunctionType.Tanh
                    )

                    # one matmul per tile-pair column: lhsT = full [128, 128]
                    # xT slice stacks both groups' hidden units on K;
                    # rhs = [[w2;0],[0;w2]] -> scores for tiles j and j+4,
                    # written via a stride-4 column view so scol stays in
                    # tile order.
                    scolv = scol[:].rearrange("p (a b) -> p b a", b=GRP)
                    for j in range(GRP):
                        nc.tensor.matmul(
                            scolv[:, j, :],
                            xT[:, j * P:(j + 1) * P],
                            w2z_sb[:],
                            start=True,
                            stop=True,
                        )
                    nc.scalar.activation(
                        e_buf[:, tp0:tp0 + PAIR],
                        scol[:],
                        mybir.ActivationFunctionType.Exp,
                    )

                    # one fused ce build for the whole pair:
                    # ce[p, k*W+w] = cmp01[p, k, w] * e[p, k]
                    c0 = PAIR * IN
                    ce = cmp_p.tile([P, PAIR * W], F16, tag="ce")
                    nc.vector.tensor_tensor(
                        ce[:].rearrange("p (k w) -> p k w", k=PAIR),
                        hp[:, c0:c0 + PAIR * W].rearrange(
                            "p (k w) -> p k w", k=PAIR),
                        e_buf[:, tp0:tp0 + PAIR][:, :, None].broadcast_to(
                            [P, PAIR, W]),
                        op=mybir.AluOpType.mult,
                    )
                    for k in range(PAIR):
                        first = (gp == 0 and k == 0)
                        last = (gp == BLK // PAIR - 1 and k == PAIR - 1)
                        nc.tensor.matmul(
                            blockp[:],
                            hp[:, k * IN:(k + 1) * IN],
                            ce[:, k * W:(k + 1) * W],
                            start=first,
                            stop=last,
                        )

                nc.vector.tensor_copy(
                    parts_buf[:, b * W:(b + 1) * W], blockp[:]
                )

            nc.sync.dma_start(e_out[:], e_buf[:])
            nc.sync.dma_start(parts[:], parts_buf[:])

    insert_act_tables(nc)
    legalize_waits(nc)
    return nc


def prep_inputs(H, w1, w2, batch, G, TPC, W, n_cores):
    """Host-side input prep. Returns (in_maps, bases, meta)."""
    N = H.shape[0]
    rows_per_core = TPC * P
    total_rows = n_cores * rows_per_core
    assert total_rows >= N, (total_rows, N)

    batch = np.asarray(batch, dtype=np.int64)
    batch_pad = np.full(total_rows, -100, dtype=np.int64)
    batch_pad[:N] = batch

    n_blocks = TPC // BLK
    block_rows = BLK * P
    bases = np.zeros((n_cores, n_blocks), dtype=np.int64)
    for c in range(n_cores):
        for b in range(n_blocks):
            r0 = c * rows_per_core + b * block_rows
            bases[c, b] = batch_pad[r0] if r0 < N else 0
    for c in range(n_cores):
        for b in range(n_blocks):
            r0 = c * rows_per_core + b * block_rows
            r1 = min(r0 + block_rows, N)
            if r0 < N:
                span = batch_pad[r0:r1].max() - bases[c, b] + 1
                assert span <= W, f"block span {span} > W={W} at core {c} block {b}"

    bsh = batch_pad - np.repeat(bases.reshape(-1), block_rows)[:total_rows]
    # host-built one-hot window indicator [total_rows, W] fp16
    cmp01 = (bsh[:, None] == np.arange(W)[None, :]).astype(np.float16)

    H_pad = np.zeros((total_rows, IN), dtype=np.float16)
    H_pad[:N] = H.astype(np.float16)

    w1T = np.ascontiguousarray(w1.T).astype(np.float16)    # [128, 64]
    # block-diagonal w2 selector: [128, 2] = [[w2, 0], [0, w2]]
    w2T = np.zeros((2 * HID, 2), dtype=np.float16)
    w2T[:HID, 0] = w2[0].astype(np.float16)
    w2T[HID:, 1] = w2[0].astype(np.float16)
    ident = np.eye(P, dtype=np.float16)

    n_pairs = TPC // PAIR
    PW = PAIR * IN + PAIR * W
    in_maps = []
    for c in range(n_cores):
        r0, r1 = c * rows_per_core, (c + 1) * rows_per_core
        Hc = H_pad[r0:r1]
        # [n_pairs, 128, PAIR*IN]: HPh[pi, p, k*128+f] = Hc[(pi*8+k)*128+p, f]
        HPh = (Hc.reshape(n_pairs, PAIR, P, IN).transpose(0, 2, 1, 3)
               .reshape(n_pairs, P, PAIR * IN))
        Cc = cmp01[r0:r1]
        CPh = (Cc.reshape(n_pairs, PAIR, P, W).transpose(0, 2, 1, 3)
               .reshape(n_pairs, P, PAIR * W))
        HP = np.ascontiguousarray(
            np.concatenate([HPh, CPh], axis=2)
        )
        in_maps.append({
            "HP": HP,
            "w1T": w1T,
            "w2T": w2T,
            "ident": ident,
        })

    meta = dict(N=N, G=G, TPC=TPC, W=W, n_cores=n_cores,
                rows_per_core=rows_per_core, n_blocks=n_blocks,
                block_rows=block_rows, batch=batch)
    return in_maps, bases, meta


def merge_outputs(results, bases, meta):
    N, G, W = meta["N"], meta["G"], meta["W"]
    n_cores, rows_per_core = meta["n_cores"], meta["rows_per_core"]
    n_blocks, block_rows = meta["n_blocks"], meta["block_rows"]
    batch = meta["batch"]
    total_rows = n_cores * rows_per_core

    out_n = np.zeros((G + 2 * W, IN), dtype=np.float64)
    e_all = np.zeros(total_rows, dtype=np.float64)
    for c in range(n_cores):
        parts = results[c]["parts"]          # [128, n_blocks*W] f32
        e_out = results[c]["e_out"]          # [128, TPC] f16
        e_all[c * rows_per_core:(c + 1) * rows_per_core] = (
            e_out.T.reshape(-1).astype(np.float64)
        )
        for b in range(n_blocks):
            g0 = int(bases[c, b])
            r0 = c * rows_per_core + b * block_rows
            if r0 >= N:
                continue
            out_n[g0:g0 + W] += parts[:, b * W:(b + 1) * W].T.astype(np.float64)

    d = np.bincount(batch, weights=e_all[:N], minlength=G)[:G]
    out_n = out_n[:G]
    with np.errstate(divide="ignore", invalid="ignore"):
        fin = np.where(d[:, None] > 0, out_n / d[:, None], 0.0)
    return fin.astype(np.float32)


def prep_and_run(H, w1, w2, batch, G, TPC=None, W=32, n_cores=8, nc=None,
                 trace=False, pack_mm1=True):
    in_maps, bases, meta = prep_inputs(H, w1, w2, batch, G, TPC, W, n_cores)
    if nc is None:
        nc = build_nc(TPC, W, pack_mm1=pack_mm1)
    res = run_bass_kernel_spmd(
        nc, in_maps, core_ids=list(range(n_cores)), trace=trace
    )
    fin = merge_outputs(res.results, bases, meta)
    return fin, res


_NC_CACHE = {}


def kernel(H, w1, w2, batch, size):
    """Full-input entry point; shards over 8 cores internally."""
    H = np.asarray(H, dtype=np.float32)
    w1 = np.asarray(w1, dtype=np.float32)
    w2 = np.asarray(w2, dtype=np.float32)
    batch = np.asarray(batch).astype(np.int64)
    G = int(np.asarray(size))
    N = H.shape[0]
    n_cores = 8

    # tiles per core: cover N with blocks of BLK tiles
    rows_needed = -(-N // n_cores)
    TPC = -(-rows_needed // (P * BLK)) * BLK

    # window: widest segment span over any block, rounded up with margin
    block_rows = BLK * P
    span = 1
    for r0 in range(0, N, block_rows):
        r1 = min(r0 + block_rows, N)
        span = max(span, int(batch[r1 - 1] - batch[r0]) + 1)
    W = 32
    while W < span:
        W *= 2
    assert W <= 512

    key = (TPC, W)
    if key not in _NC_CACHE:
        _NC_CACHE[key] = build_nc(TPC, W)
    fin, _ = prep_and_run(H, w1, w2, batch, G, TPC=TPC, W=W,
                          n_cores=n_cores, nc=_NC_CACHE[key])
    return fin



# revision 24
# speedup vs baseline: 2.1933x; 1.0129x over previous
"""Trainium2 Bass kernel for AttnPooling (segment softmax pooling).

kernel(**inputs) takes the FULL inputs (H [N,128] f32, w1 [64,128],
w2 [1,64], batch [N] int64 sorted, size scalar) and returns the FULL
[size, 128] f32 output, computed on 8 NeuronCores via bass/Tile.

Self-contained: includes the wait-legalization/ACT-table/NTFF shims the
plain-Bass + TileContext + walrus flow needs in this environment.
"""

import contextlib
import ctypes
import sys
import types

import numpy as np

import concourse.bass as bass
import concourse.tile as tile
from concourse import mybir
from concourse.bass_utils import run_bass_kernel_spmd


# ---------------------------------------------------------------------------
# environment shims
def legalize_waits(nc):
    br = bass._bass_rust
    try:
        br.move_matmul_waits_to_ldweights(nc.m)
    except Exception:
        pass
    br.generate_event_semaphores(nc)
    return nc


def insert_act_tables(nc):
    has_activation = any(
        isinstance(i, mybir.InstActivation)
        for b in nc.main_func.blocks
        for i in b.instructions
    )
    if not has_activation:
        return
    from concourse.hw_specs import get_activation_tables
    tables = list(get_activation_tables(nc.m.arch).items())
    bass._bass_rust.insert_act_table_loads(nc, tables)


def install_ntff_hook(so_path="/opt/axon/libaxon_pjrt.so"):
    if "antenv.axon_hooks" in sys.modules:
        return
    try:
        lib = ctypes.CDLL(so_path)
    except OSError:
        return
    if not hasattr(lib, "axon_start_nrt_profile"):
        return
    lib.axon_start_nrt_profile.argtypes = [
        ctypes.POINTER(ctypes.c_int64), ctypes.c_size_t]
    lib.axon_start_nrt_profile.restype = ctypes.c_int64
    lib.axon_stop_nrt_profile.argtypes = [ctypes.c_char_p]
    lib.axon_stop_nrt_profile.restype = ctypes.c_int64

    @contextlib.contextmanager
    def _hook(output_dir, device_ids):
        import jax
        jax.devices()
        if device_ids:
            ids = (ctypes.c_int64 * len(device_ids))(*device_ids)
            rc = lib.axon_start_nrt_profile(ids, len(device_ids))
        else:
            rc = lib.axon_start_nrt_profile(None, 0)
        if rc != 0:
            raise RuntimeError(f"axon_start_nrt_profile rc={rc}")
        try:
            yield
        finally:
            n = lib.axon_stop_nrt_profile(str(output_dir).encode())
            print(f"ntff profile: {n} file(s) written to {output_dir}",
                  file=sys.stderr)

    mod = types.ModuleType("antenv.axon_hooks")
    mod._hook = _hook
    mod.get_axon_ntff_profile_hook = lambda: _hook
    mod.set_axon_ntff_profile_hook = lambda h: None
    sys.modules["antenv.axon_hooks"] = mod


install_ntff_hook()


F32 = mybir.dt.float32
F16 = mybir.dt.float16

P = 128
IN = 128
HID = 64
BLK = 16          # tiles per block
GRP = 4           # tiles per MLP group
PAIR = 2 * GRP    # tiles per DMA / tanh / exp batch


def build_nc(TPC: int, W: int, pack_mm1: bool = True):
    assert TPC % BLK == 0
    n_blocks = TPC // BLK
    n_pairs = TPC // PAIR
    PW = PAIR * IN + PAIR * W      # fp16 columns per pair payload
    nc = bass.Bass("TRN2", target_bir_lowering=False, debug=False)

    HP = nc.dram_tensor("HP", [n_pairs, P, PW], F16, kind="ExternalInput")
    w1T = nc.dram_tensor("w1T", [IN, HID], F16, kind="ExternalInput")
    w2T = nc.dram_tensor("w2T", [2 * HID, 2], F16, kind="ExternalInput")
    ident = nc.dram_tensor("ident", [P, P], F16, kind="ExternalInput")
    parts = nc.dram_tensor("parts", [P, n_blocks * W], F32,
                           kind="ExternalOutput")
    e_out = nc.dram_tensor("e_out", [P, TPC], F16, kind="ExternalOutput")

    with tile.TileContext(nc) as tc:
        with (
            tc.tile_pool(name="consts", bufs=1) as consts,
            tc.tile_pool(name="hin", bufs=8) as hin,
            tc.tile_pool(name="ht", bufs=3) as htp,
            tc.tile_pool(name="xt", bufs=3) as xtp,
            tc.tile_pool(name="cmp", bufs=8) as cmp_p,
            tc.tile_pool(name="ps_tr", bufs=2, space="PSUM") as ps_tr,
            tc.tile_pool(name="ps_mlp", bufs=2, space="PSUM") as ps_mlp,
            tc.tile_pool(name="ps_s", bufs=1, space="PSUM") as ps_s,
            tc.tile_pool(name="ps_blk", bufs=1, space="PSUM") as ps_blk,
        ):
            w1T_sb = consts.tile([IN, HID], F16)
            nc.sync.dma_start(w1T_sb[:], w1T[:])
            w2z_sb = consts.tile([2 * HID, 2], F16)
            nc.sync.dma_start(w2z_sb[:], w2T[:])
            ident_sb = consts.tile([P, P], F16)
            nc.sync.dma_start(ident_sb[:], ident[:])
            e_buf = consts.tile([P, TPC], F16)
            parts_buf = consts.tile([P, n_blocks * W], F32)

            # PE warmup: ~3.5us of dense matmul activity releases the HAM
            # clock throttle (1.2 -> 2.4 GHz) before the steady state.
            warm = ps_tr.tile([P, 2 * GRP * P], F32, tag="trp")
            for _ in range(48):
                nc.tensor.matmul(warm[:, :P], ident_sb[:], ident_sb[:],
                                 start=True, stop=True)

            for b in range(n_blocks):
                blockp = ps_blk.tile([P, W], F32, tag="blockp")
                for gp in range(BLK // PAIR):
                    pi = b * (BLK // PAIR) + gp
                    tp0 = b * BLK + gp * PAIR
                    hp = hin.tile([P, PW], F16, tag="hin")
                    nc.sync.dma_start(hp[:], HP[pi])

                    mlp = ps_mlp.tile([2 * HID, GRP * P], F32, tag="mlp")
                    scol = ps_s.tile([P, PAIR], F32, tag="scol")
                    # both groups' transposes land in one 2-bank psum tile;
                    # ONE strided ACT copy + ONE strided DVE cast each span
                    # both groups, so the two col-packed mm1s share an
                    # identical dependency set and issue back-to-back
                    # (concurrent h0/h64 execution), with 4->2 copy
                    # instructions saving fixed overhead.
                    trp2 = ps_tr.tile([P, 2 * GRP * P], F32, tag="trp",
                                      name="trp2")
                    HT2 = htp.tile([P, 2 * GRP * P], F16, tag="HT",
                                   name="HT2")
                    for g in range(2):
                        for j in range(GRP):
                            t_off = (g * GRP + j) * IN
                            tp = (g * GRP + j) * P
                            nc.tensor.matmul(
                                trp2[:, tp:tp + P],
                                hp[:, t_off:t_off + IN],
                                ident_sb[:],
                                start=True,
                                stop=True,
                            )
                    act_cols = 160
                    t2v = trp2[:].rearrange("p (g c) -> p g c", g=2)
                    h2v = HT2[:].rearrange("p (g c) -> p g c", g=2)
                    nc.scalar.copy(h2v[:, :, :act_cols],
                                   t2v[:, :, :act_cols])
                    nc.vector.tensor_copy(h2v[:, :, act_cols:],
                                          t2v[:, :, act_cols:])

                    for g in range(2):
                        pos = (0, g * HID) if pack_mm1 else None
                        nc.tensor.matmul(
                            mlp[g * HID:(g + 1) * HID, :],
                            w1T_sb[:],
                            HT2[:, g * GRP * P:(g + 1) * GRP * P],
                            start=True,
                            stop=True,
                            tile_position=pos,
                        )

                    xT = xtp.tile([2 * HID, GRP * P], F16, tag="xT")
                    nc.scalar.activation(
                        xT[:], mlp[:], mybir.ActivationFunctionType.Tanh
                    )

                    # one matmul per tile-pair column: lhsT = full [128, 128]
                    # xT slice stacks both groups' hidden units on K;
                    # rhs = [[w2;0],[0;w2]] -> scores for tiles j and j+4,
                    # written via a stride-4 column view so scol stays in
                    # tile order.
                    scolv = scol[:].rearrange("p (a b) -> p b a", b=GRP)
                    for j in range(GRP):
                        nc.tensor.matmul(
                            scolv[:, j, :],
                            xT[:, j * P:(j + 1) * P],
                            w2z_sb[:],
                            start=True,
                            stop=True,
                        )
                    nc.scalar.activation(
                        e_buf[:, tp0:tp0 + PAIR],
                        scol[:],
                        mybir.ActivationFunctionType.Exp,
                    )

                    # one fused ce build for the whole pair:
                    # ce[p, k*W+w] = cmp01[p, k, w] * e[p, k]
                    c0 = PAIR * IN
                    ce = cmp_p.tile([P, PAIR * W], F16, tag="ce")
                    nc.vector.tensor_tensor(
                        ce[:].rearrange("p (k w) -> p k w", k=PAIR),
                        hp[:, c0:c0 + PAIR * W].rearrange(
                            "p (k w) -> p k w", k=PAIR),
                        e_buf[:, tp0:tp0 + PAIR][:, :, None].broadcast_to(
                            [P, PAIR, W]),
                        op=mybir.AluOpType.mult,
                    )
                    for k in range(PAIR):
                        first = (gp == 0 and k == 0)
                        last = (gp == BLK // PAIR - 1 and k == PAIR - 1)
                        nc.tensor.matmul(
                            blockp[:],
                            hp[:, k * IN:(k + 1) * IN],
                            ce[:, k * W:(k + 1) * W],
                            start=first,
                            stop=last,
                        )

                nc.vector.tensor_copy(
                    parts_buf[:, b * W:(b + 1) * W], blockp[:]
                )

            nc.sync.dma_start(e_out[:], e_buf[:])
            nc.sync.dma_start(parts[:], parts_buf[:])

    insert_act_tables(nc)
    legalize_waits(nc)
    return nc


def prep_inputs(H, w1, w2, batch, G, TPC, W, n_cores):
    """Host-side input prep. Returns (in_maps, bases, meta)."""
    N = H.shape[0]
    rows_per_core = TPC * P
    total_rows = n_cores * rows_per_core
    assert total_rows >= N, (total_rows, N)

    batch = np.asarray(batch, dtype=np.int64)
    batch_pad = np.full(total_rows, -100, dtype=np.int64)
    batch_pad[:N] = batch

    n_blocks = TPC // BLK
    block_rows = BLK * P
    bases = np.zeros((n_cores, n_blocks), dtype=np.int64)
    for c in range(n_cores):
        for b in range(n_blocks):
            r0 = c * rows_per_core + b * block_rows
            bases[c, b] = batch_pad[r0] if r0 < N else 0
    for c in range(n_cores):
        for b in range(n_blocks):
            r0 = c * rows_per_core + b * block_rows
            r1 = min(r0 + block_rows, N)
            if r0 < N:
                span = batch_pad[r0:r1].max() - bases[c, b] + 1
                assert span <= W, f"block span {span} > W={W} at core {c} block {b}"

    bsh = batch_pad - np.repeat(bases.reshape(-1), block_rows)[:total_rows]
    # host-built one-hot window indicator [total_rows, W] fp16
    cmp01 = (bsh[:, None] == np.arange(W)[None, :]).astype(np.float16)

    H_pad = np.zeros((total_rows, IN), dtype=np.float16)
    H_pad[:N] = H.astype(np.float16)

    w1T = np.ascontiguousarray(w1.T).astype(np.float16)    # [128, 64]
    # block-diagonal w2 selector: [128, 2] = [[w2, 0], [0, w2]]
    w2T = np.zeros((2 * HID, 2), dtype=np.float16)
    w2T[:HID, 0] = w2[0].astype(np.float16)
    w2T[HID:, 1] = w2[0].astype(np.float16)
    ident = np.eye(P, dtype=np.float16)

    n_pairs = TPC // PAIR
    PW = PAIR * IN + PAIR * W
    in_maps = []
    for c in range(n_cores):
        r0, r1 = c * rows_per_core, (c + 1) * rows_per_core
        Hc = H_pad[r0:r1]
        # [n_pairs, 128, PAIR*IN]: HPh[pi, p, k*128+f] = Hc[(pi*8+k)*128+p, f]
        HPh = (Hc.reshape(n_pairs, PAIR, P, IN).transpose(0, 2, 1, 3)
               .reshape(n_pairs, P, PAIR * IN))
        Cc = cmp01[r0:r1]
        CPh = (Cc.reshape(n_pairs, PAIR, P, W).transpose(0, 2, 1, 3)
               .reshape(n_pairs, P, PAIR * W))
        HP = np.ascontiguousarray(
            np.concatenate([HPh, CPh], axis=2)
        )
        in_maps.append({
            "HP": HP,
            "w1T": w1T,
            "w2T": w2T,
            "ident": ident,
        })

    meta = dict(N=N, G=G, TPC=TPC, W=W, n_cores=n_cores,
                rows_per_core=rows_per_core, n_blocks=n_blocks,
                block_rows=block_rows, batch=batch)
    return in_maps, bases, meta


def merge_outputs(results, bases, meta):
    N, G, W = meta["N"], meta["G"], meta["W"]
    n_cores, rows_per_core = meta["n_cores"], meta["rows_per_core"]
    n_blocks, block_rows = meta["n_blocks"], meta["block_rows"]
    batch = meta["batch"]
    total_rows = n_cores * rows_per_core

    out_n = np.zeros((G + 2 * W, IN), dtype=np.float64)
    e_all = np.zeros(total_rows, dtype=np.float64)
    for c in range(n_cores):
        parts = results[c]["parts"]          # [128, n_blocks*W] f32
        e_out = results[c]["e_out"]          # [128, TPC] f16
        e_all[c * rows_per_core:(c + 1) * rows_per_core] = (
            e_out.T.reshape(-1).astype(np.float64)
        )
        for b in range(n_blocks):
            g0 = int(bases[c, b])
            r0 = c * rows_per_core + b * block_rows
            if r0 >= N:
                continue
            out_n[g0:g0 + W] += parts[:, b * W:(b + 1) * W].T.astype(np.float64)

    d = np.bincount(batch, weights=e_all[:N], minlength=G)[:G]
    out_n = out_n[:G]
    with np.errstate(divide="ignore", invalid="ignore"):
        fin = np.where(d[:, None] > 0, out_n / d[:, None], 0.0)
    return fin.astype(np.float32)


def prep_and_run(H, w1, w2, batch, G, TPC=None, W=32, n_cores=8, nc=None,
                 trace=False, pack_mm1=True):
    in_maps, bases, meta = prep_inputs(H, w1, w2, batch, G, TPC, W, n_cores)
    if nc is None:
        nc = build_nc(TPC, W, pack_mm1=pack_mm1)
    res = run_bass_kernel_spmd(
        nc, in_maps, core_ids=list(range(n_cores)), trace=trace
    )
    fin = merge_outputs(res.results, bases, meta)
    return fin, res


_NC_CACHE = {}


def kernel(H, w1, w2, batch, size):
    """Full-input entry point; shards over 8 cores internally."""
    H = np.asarray(H, dtype=np.float32)
    w1 = np.asarray(w1, dtype=np.float32)
    w2 = np.asarray(w2, dtype=np.float32)
    batch = np.asarray(batch).astype(np.int64)
    G = int(np.asarray(size))
    N = H.shape[0]
    n_cores = 8

    # tiles per core: cover N with blocks of BLK tiles
    rows_needed = -(-N // n_cores)
    TPC = -(-rows_needed // (P * BLK)) * BLK

    # window: widest segment span over any block, rounded up with margin
    block_rows = BLK * P
    span = 1
    for r0 in range(0, N, block_rows):
        r1 = min(r0 + block_rows, N)
        span = max(span, int(batch[r1 - 1] - batch[r0]) + 1)
    W = 32
    while W < span:
        W *= 2
    assert W <= 512

    key = (TPC, W)
    if key not in _NC_CACHE:
        _NC_CACHE[key] = build_nc(TPC, W)
    fin, _ = prep_and_run(H, w1, w2, batch, G, TPC=TPC, W=W,
                          n_cores=n_cores, nc=_NC_CACHE[key])
    return fin

